# revision 16
# baseline (speedup 1.0000x reference)
"""LoRA linear (y = x @ (W + s*B@A)^T + bias) on 8 Trainium2 NeuronCores.

Strategy: pure data parallel over the token dim; LoRA folded into the weight
on the host (W' = W + 4 * B @ A); x / W' / bias cast to bf16 on the host
(fro rel-err ~3e-3, far inside the 2e-2 gate) which halves all DMA traffic
and keeps the PE at 1 col/cycle (same rate as fp32r, half the bytes).

Per core: out[2048, 1024](bf16) = x @ W'^T + bias, x/W resident in SBUF.

The host pre-arranges the operands into partition-major layouts that match
SBUF exactly, so the startup stream is 14 large fully-contiguous DMAs in
consumption order on the single sync HWDGE ring (every dma_start costs
~0.62 us of sequencer issue time, and a DMA's completion sem only fires on
full completion — issue count and sem granularity both gate the pipeline):
  xA [128, 8*512]   tokens 0:512 of each d-block (group 0, streamed first)
  xB [128, 8*1536]  tokens 512:2048 (jobs; first 512-token block separate)
  wH [128, 8*1024]  weight in SBUF layout
  bias [128, 1024]  pre-replicated

Compute structure:
  - ~26 N=128 warm-up matmuls on zeroed scratch bridge the engine preamble
    (~7.3 us) to operand arrival (~10 us) so the HAM clock gate reaches 8/8
    with no PE idle gap (else the first ~3.4 us of real matmuls run at
    1.2 GHz instead of 2.4).
  - PSUM as 8 single-bank tiles [128(n), 512(o)].
    Group 0 (tokens 0:512) runs d-outer so each arriving (w[d], x[d]) pair
    immediately enables 8 matmuls.
    Tokens 512:2048 run as a job pipeline: one job = (128 tokens, 512 outs,
    all 8 d) = 8 matmuls into one bank, then evict. Evictions are spaced
    every ~1.7 us instead of bursting 8 at each group boundary, so a bank
    is always free ~13 us before reuse -> no PE stalls.
  - Evictions on DVE (GPSIMD cannot read PSUM): psum fp32 + bias bf16 ->
    bf16 out, then out DMA on the sync ring (input stream is done before
    output volume matters). The last job runs as two N=256 chains so the
    exposed tail is a single [128,256] add + small DMA.
"""

import os
import sys

import numpy as np

for _p in ("/opt/trn_rl_repo", "/opt/pypackages"):
    if os.path.isdir(_p) and _p not in sys.path:
        sys.path.append(_p)

try:
    import jax

    jax.config.update(
        "jax_compilation_cache_dir", os.path.expanduser("~/.cache/jax_bass_cache")
    )
    jax.config.update("jax_persistent_cache_min_compile_time_secs", 0.0)
except Exception:
    pass

try:
    # bass_utils imports this when tracing is requested via BASS_TRACE; the
    # agent image ships a stub antenv without it. Register a no-op fallback
    # so a trace request degrades to "no trace" instead of crashing.
    from antenv import axon_hooks as _axon_hooks  # noqa: F401
except ImportError:
    import types as _types

    import antenv as _antenv

    _hooks = _types.ModuleType("antenv.axon_hooks")
    _hooks._hook = None
    _hooks.set_axon_ntff_profile_hook = lambda h: setattr(_hooks, "_hook", h)
    _hooks.get_axon_ntff_profile_hook = lambda: _hooks._hook
    sys.modules["antenv.axon_hooks"] = _hooks
    _antenv.axon_hooks = _hooks

import ml_dtypes  # noqa: E402

import concourse.bass as bass  # noqa: E402,F401
import concourse.mybir as mybir  # noqa: E402
import concourse.tile as tile  # noqa: E402
from concourse import bacc  # noqa: E402
from concourse.bass_utils import run_bass_kernel_spmd  # noqa: E402

N_CORES = 8
N_TOK, D_IN, D_OUT = 16384, 1024, 1024
N_SHARD = N_TOK // N_CORES  # 2048 tokens per core
P = 128
KT = D_IN // P  # 8 contraction (d) blocks
NBLK = 512  # tokens per group
TB = N_SHARD - NBLK  # 1536 tail tokens per d
SCALING = 4.0  # alpha / r = 32 / 8

_CACHE: dict = {}


def build_nc():
    f32 = mybir.dt.float32
    bf16 = mybir.dt.bfloat16
    nc = bacc.Bacc("TRN2", target_bir_lowering=False, debug=False)

    xA = nc.dram_tensor("xA", [P, KT * NBLK], bf16, kind="ExternalInput")
    xB = nc.dram_tensor("xB", [P, KT * TB], bf16, kind="ExternalInput")
    wH = nc.dram_tensor("wH", [P, KT * D_OUT], bf16, kind="ExternalInput")
    bias = nc.dram_tensor("bias", [P, D_OUT], bf16, kind="ExternalInput")
    out = nc.dram_tensor("out", [N_SHARD, D_OUT], bf16, kind="ExternalOutput")

    OH = 512  # one PSUM bank of fp32 output per matmul
    NH = D_OUT // OH  # 2 output halves

    with tile.TileContext(nc) as tc:
        with tc.tile_pool(name="const", bufs=1) as const_pool, \
                tc.tile_pool(name="ps", bufs=8, space="PSUM") as psum_pool:
            xA_sb = const_pool.tile([P, KT * NBLK], bf16, name="xA_sb")
            xB_sb = const_pool.tile([P, KT * TB], bf16, name="xB_sb")
            w_sb = const_pool.tile([P, KT * D_OUT], bf16, name="w_sb")
            bias_sb = const_pool.tile([P, D_OUT], bf16, name="bias_sb")

            def xsl(kt, t0, t1):
                # token range [t0, t1) of d-block kt; never crosses the 512
                # boundary (group 0 uses xA, jobs use xB).
                if t1 <= NBLK:
                    return xA_sb[:, kt * NBLK + t0:kt * NBLK + t1]
                assert t0 >= NBLK
                return xB_sb[:, kt * TB + t0 - NBLK:kt * TB + t1 - NBLK]

            def wsl(kt, o0, o1):
                return w_sb[:, kt * D_OUT + o0:kt * D_OUT + o1]

            # Warm-up scratch (zeroed so the PE never streams NaN garbage).
            warm_x = const_pool.tile([P, P], bf16, name="warm_x")
            warm_w = const_pool.tile([P, OH], bf16, name="warm_w")
            nc.gpsimd.memset(warm_x[:], 0.0)
            nc.gpsimd.memset(warm_w[:], 0.0)
            warm_ps = psum_pool.tile([P, OH], f32, name="warm_ps", tag="psum")
            for _ in range(26):
                nc.tensor.matmul(warm_ps[:, 0:P], warm_x[:], warm_w[:, 0:P],
                                 start=True, stop=True)

            # Startup stream: 14 contiguous DMAs in consumption order.
            def ld(dst, src, c0, c1):
                nc.sync.dma_start(dst[:, c0:c1], src[:, c0:c1])

            ld(xA_sb, xA, 0, NBLK)                    # x d0        128 KB
            ld(w_sb, wH, 0, OH)                       # w d0 h0     128 KB
            ld(w_sb, wH, OH, D_OUT)                   # w d0 h1     128 KB
            ld(xA_sb, xA, NBLK, 2 * NBLK)             # x d1        128 KB
            ld(w_sb, wH, D_OUT, 2 * D_OUT)            # w d1        256 KB
            ld(xA_sb, xA, 2 * NBLK, 4 * NBLK)         # x d2-d3     256 KB
            ld(w_sb, wH, 2 * D_OUT, 4 * D_OUT)        # w d2-d3     512 KB
            ld(xA_sb, xA, 4 * NBLK, 6 * NBLK)         # x d4-d5     256 KB
            ld(w_sb, wH, 4 * D_OUT, 6 * D_OUT)        # w d4-d5     512 KB
            ld(xA_sb, xA, 6 * NBLK, 8 * NBLK)         # x d6-d7     256 KB
            ld(w_sb, wH, 6 * D_OUT, 8 * D_OUT)        # w d6-d7     512 KB
            nc.sync.dma_start(bias_sb[:], bias[:])    # bias        256 KB
            # Tails: tokens 512:1024 of every d first (jobs 0-7 need all
            # eight d-slices and a DMA's sem fires only on completion),
            # then the rest.
            for t in range(KT):
                ld(xB_sb, xB, t * TB, t * TB + NBLK)        # 8 x 128 KB
            for t in range(KT):
                ld(xB_sb, xB, t * TB + NBLK, (t + 1) * TB)  # 8 x 256 KB

            def evict(n0, h, psum, o_sb):
                nc.vector.tensor_add(o_sb[:, h * OH:(h + 1) * OH], psum[:],
                                     bias_sb[:, h * OH:(h + 1) * OH])
                nc.sync.dma_start(out[n0:n0 + P, h * OH:(h + 1) * OH],
                                  o_sb[:, h * OH:(h + 1) * OH])

            # ---- Group 0 (tokens 0:512): d-outer over 8 single-bank psums.
            g0_ps = [
                psum_pool.tile([P, OH], f32, name=f"ps_g0_{i}_{h}", tag="psum")
                for i in range(4) for h in range(NH)
            ]
            g0_osb = [const_pool.tile([P, D_OUT], bf16, name=f"o_g0_{i}")
                      for i in range(4)]
            for d in range(KT):
                for i in range(4):
                    lhsT = xsl(d, i * P, (i + 1) * P)
                    for h in range(NH):
                        nc.tensor.matmul(
                            g0_ps[i * NH + h][:],
                            lhsT,
                            wsl(d, h * OH, (h + 1) * OH),
                            start=(d == 0),
                            stop=(d == KT - 1),
                        )
            for i in range(4):
                for h in range(NH):
                    evict(i * P, h, g0_ps[i * NH + h], g0_osb[i])

            # ---- Tokens 512:2048: job pipeline. One job = (128 tokens,
            # 512 outs, all 8 d) into one psum bank, then evict.
            jobs = [
                (NBLK + j // NH * P, j % NH)  # (token offset, out half)
                for j in range(((N_SHARD - NBLK) // P) * NH)
            ]
            n_jobs = len(jobs)
            osb_map = {}
            for j, (n0, h) in enumerate(jobs):
                if h == 0:
                    osb_map[n0] = const_pool.tile([P, D_OUT], bf16,
                                                  name=f"o_j{n0}")
                ps = psum_pool.tile([P, OH], f32, name=f"ps_j{j}", tag="psum")
                if j < n_jobs - 1:
                    for d in range(KT):
                        nc.tensor.matmul(
                            ps[:],
                            xsl(d, n0, n0 + P),
                            wsl(d, h * OH, (h + 1) * OH),
                            start=(d == 0),
                            stop=(d == KT - 1),
                        )
                    evict(n0, h, ps, osb_map[n0])
                else:
                    # Last job: two independent N=256 chains so the first
                    # half evicts ~0.9 us before the final matmul and the
                    # exposed tail is a single [128,256] add + DMA.
                    Q = OH // 2
                    o_sb = osb_map[n0]
                    for cq in range(2):
                        o0 = h * OH + cq * Q
                        for d in range(KT):
                            nc.tensor.matmul(
                                ps[:, cq * Q:(cq + 1) * Q],
                                xsl(d, n0, n0 + P),
                                wsl(d, o0, o0 + Q),
                                start=(d == 0),
                                stop=(d == KT - 1),
                            )
                        nc.vector.tensor_add(
                            o_sb[:, o0:o0 + Q],
                            ps[:, cq * Q:(cq + 1) * Q],
                            bias_sb[:, o0:o0 + Q],
                        )
                        nc.sync.dma_start(out[n0:n0 + P, o0:o0 + Q],
                                          o_sb[:, o0:o0 + Q])

    nc.finalize()
    return nc


def _get_nc():
    if "nc" not in _CACHE:
        _CACHE["nc"] = build_nc()
    return _CACHE["nc"]


def kernel(x, weight, bias, A, B):
    x = np.asarray(x, dtype=np.float32)
    weight = np.asarray(weight, dtype=np.float32)
    bias = np.asarray(bias, dtype=np.float32)
    A = np.asarray(A, dtype=np.float32)
    B = np.asarray(B, dtype=np.float32)

    # Fold the rank-8 LoRA update into the weight (exact up to rounding).
    w_eff = (
        weight.astype(np.float64)
        + SCALING * (B.astype(np.float64) @ A.astype(np.float64))
    ).astype(np.float32)
    # Weight in SBUF layout: (p, d*1024 + o) = W'^T[d*128 + p, o].
    wh = np.ascontiguousarray(
        w_eff.T.astype(ml_dtypes.bfloat16)
        .reshape(KT, P, D_OUT).transpose(1, 0, 2).reshape(P, KT * D_OUT)
    )
    bias_rep = np.ascontiguousarray(
        np.broadcast_to(bias.astype(ml_dtypes.bfloat16), (P, D_OUT))
    )
    xT = x.T.astype(ml_dtypes.bfloat16)  # [d, n]

    nc = _get_nc()
    in_maps = []
    for c in range(N_CORES):
        shard = np.ascontiguousarray(
            xT[:, c * N_SHARD:(c + 1) * N_SHARD]
        ).reshape(KT, P, N_SHARD)
        xa = np.ascontiguousarray(
            shard[:, :, 0:NBLK].transpose(1, 0, 2).reshape(P, KT * NBLK)
        )
        xb = np.ascontiguousarray(
            shard[:, :, NBLK:].transpose(1, 0, 2).reshape(P, KT * TB)
        )
        in_maps.append({"xA": xa, "xB": xb, "wH": wh, "bias": bias_rep})
    trace_kwargs = {}
    if os.environ.get("KERNEL_TRACE") == "1":
        trace_kwargs = {"trace": True}
    res = run_bass_kernel_spmd(nc, in_maps, list(range(N_CORES)), **trace_kwargs)
    _CACHE["last_results"] = res
    return np.concatenate(
        [r["out"].astype(np.float32) for r in res.results], axis=0
    )
